# revision 35
# baseline (speedup 1.0000x reference)
"""Trainium2 Bass kernel for nn_AdaptiveRouter (MoE top-2 routing).

Computes, for x [16384, 2048], gate_w [2048, 64], expert_loads [64]:
  logits = x @ gate_w
  top2 idx of (logits + bias), bias = -(loads - 1/64)*2
  routing_weights = softmax of the selected unbiased logits
  new_loads = 0.9*loads + 0.1*segment_sum(weights)/T

Strategy: data-parallel over tokens across 8 NeuronCores. Host pre-transposes
x to [D, T] layout and splits f32 into bf16 hi+lo pairs (exact to ~2^-17) so
the gate matmul runs as three bf16 matmuls at full PE rate with the
contraction dim on partitions (same total DMA bytes as f32). On device:
matmul -> logits^T [64, T] in PSUM, PE-transpose back to [T, 64] tiles,
top-2 via DVE max8/max_index, softmax via sigmoid of the logit difference
(bias gathered by one-hot masks), per-expert load sums via tiny PE matmuls
contracting the token-partition axis. The 8 per-core load partials are
summed on the host during unsharding (the all-reduce of the hint).
"""

import os
import sys

for _p in ("/opt/trn_rl_repo", "/root/.axon_site/_ro/trn_rl_repo"):
    if os.path.isdir(_p) and _p not in sys.path:
        sys.path.insert(0, _p)

import numpy as np
import ml_dtypes

BF16 = ml_dtypes.bfloat16

T, D, E = 16384, 2048, 64
NCORES = 8
TS = T // NCORES          # tokens per core shard = 2048
KBLK = D // 128           # 16 contraction blocks
NTB = TS // 128           # 16 token blocks per core
NCH = TS // 512           # 4 psum chunks per core
SMOOTHING = 0.9

_RUNNER_CACHE = {}

# Bumped on every kernel-graph change: the NEFF compile cache keys on the HLO,
# which does NOT encode the bass graph (only the I/O signature) — a unique
# salt-input shape per version prevents stale-NEFF cache hits.
KERNEL_VERSION = 20


def _salt_len(mode, iters, seg_split=True):
    return (KERNEL_VERSION * 131 + len(mode) * 7 + (iters % 97)
            + (0 if seg_split else 499)) % 997 + 1


def build_nc(mode="bf16x3", iters=1, seg_split=True):
    import concourse.bacc as bacc
    import concourse.mybir as mybir
    from concourse import tile

    f32 = mybir.dt.float32
    bf16 = mybir.dt.bfloat16
    i32 = mybir.dt.int32
    u32 = mybir.dt.uint32
    Alu = mybir.AluOpType
    Act = mybir.ActivationFunctionType

    split = mode == "bf16x3"
    nterm = 3 if split else 1    # matmul terms: gh@xh, gl@xh, gh@xl
    nxp = 2 if split else 1      # x planes: xh, xl
    gw_m = 128 if split else E   # gws packs [gh | gl] along the last axis
    xdt = bf16 if split else f32
    # xs DRAM layout: [NCH, 128, KBLK, nxp, 512] (chunk-major over tokens,
    # partition-contiguous so one chunk = one large efficient DMA)
    nc = bacc.Bacc("TRN2", target_bir_lowering=False, debug=False, num_devices=NCORES)
    xs = nc.dram_tensor("xs", [NCH, 128, KBLK, nxp, 512], xdt, kind="ExternalInput")
    salt = nc.dram_tensor("salt", [1, _salt_len(mode, iters, seg_split)], f32,
                          kind="ExternalInput")
    gws = nc.dram_tensor("gws", [128, KBLK, gw_m], xdt, kind="ExternalInput")
    bias_bc = nc.dram_tensor("bias_bc", [128, E], f32, kind="ExternalInput")
    ident = nc.dram_tensor("ident", [128, 64], f32, kind="ExternalInput")
    w_out = nc.dram_tensor("w_out", [128, 2 * NTB], f32, kind="ExternalOutput")
    i_out = nc.dram_tensor("i_out", [128, 2 * NTB], i32, kind="ExternalOutput")
    loads_out = nc.dram_tensor("loads_out", [1, E], f32, kind="ExternalOutput")

    with tile.TileContext(nc) as tc:
        from contextlib import ExitStack

        ctx = ExitStack()
        const = ctx.enter_context(tc.tile_pool(name="const", bufs=1))
        xpool = ctx.enter_context(tc.tile_pool(name="xpool", bufs=4))
        work = ctx.enter_context(tc.tile_pool(name="work", bufs=1))
        pmain = ctx.enter_context(tc.tile_pool(name="pmain", bufs=1, space="PSUM"))
        ptp = ctx.enter_context(tc.tile_pool(name="ptp", bufs=2, space="PSUM"))

        # ---- constants ----
        gws_sb = const.tile([128, KBLK, gw_m], xdt)
        nc.sync.dma_start(out=gws_sb[:], in_=gws.ap())
        bias_sb = const.tile([128, E], f32)
        nc.sync.dma_start(out=bias_sb[:], in_=bias_bc.ap())
        ident_sb = const.tile([128, 64], f32)
        nc.sync.dma_start(out=ident_sb[:], in_=ident.ap())
        iota_i = const.tile([128, E], i32)
        nc.gpsimd.iota(iota_i[:], pattern=[[1, E]], base=0, channel_multiplier=0)
        iota_f = const.tile([128, E], f32)
        nc.vector.tensor_copy(iota_f[:], iota_i[:])


        def body():
            # whole-shard accumulators, written chunk-slice-wise
            biased = work.tile([128, NTB * E], f32)
            mx = work.tile([128, NTB * 8], f32)
            mi = work.tile([128, NTB * 8], u32)
            idxf = work.tile([128, 2, NTB], f32)
            bd = work.tile([128, NTB], f32)
            dlt = work.tile([128, NTB], f32)
            w_sb = work.tile([128, 2 * NTB], f32)
            i_sb = work.tile([128, 2 * NTB], i32)
            loads_ps = pmain.tile([1, E], f32, name="loads_ps")

            mi3 = mi[:].rearrange("p (b s) -> p b s", s=8)
            mx3 = mx[:].rearrange("p (b s) -> p b s", s=8)
            w3 = w_sb[:].rearrange("p (b k) -> p b k", k=2)

            def bcast3(t, nb):
                return t[:].rearrange("p (o e) -> p o e", o=1).broadcast_to(
                    (128, nb, E)
                )

            DQ = 4  # dblks per DMA (~1MB each, 8KB contiguous per partition)
            # All x DMAs are issued up-front into resident tiles: the SDMA
            # queue streams continuously and matmuls chase the arriving quads.
            xtiles = {}
            for c in range(NCH):
                for q in range(KBLK // DQ):
                    xt = xpool.tile([128, DQ, nxp, 512], xdt,
                                    name=f"x{c}_{q}", bufs=1)
                    nc.sync.dma_start(
                        out=xt[:], in_=xs.ap()[c, :, q * DQ:(q + 1) * DQ]
                    )
                    xtiles[(c, q)] = xt

            # Segments: full 512-token chunks, except the last chunk is split
            # into two 256-token halves so the final exposed tail is shorter.
            if seg_split:
                segs = [(c, 0, 512) for c in range(NCH - 1)]
                segs += [(NCH - 1, 0, 256), (NCH - 1, 256, 256)]
            else:
                segs = [(c, 0, 512) for c in range(NCH)]
            for si, (c, t0, tw) in enumerate(segs):
                ntb = tw // 128
                tb0 = (c * 512 + t0) // 128
                last_seg = si == len(segs) - 1

                # ---- gate matmul for this token segment ----
                lg_ps = pmain.tile([E, tw], f32, name=f"lg_ps{tw}", bufs=2)
                for dblk in range(KBLK):
                    for term in range(nterm):
                        # term 0: gh@xh, term 1: gl@xh, term 2: gh@xl
                        g_lo = E if term == 1 else 0
                        x_i = 1 if term == 2 else 0
                        rhs = xtiles[(c, dblk // DQ)][:, dblk % DQ, x_i,
                                                      t0:t0 + tw]
                        nc.tensor.matmul(
                            lg_ps[:],
                            lhsT=gws_sb[:, dblk, g_lo:g_lo + E],
                            rhs=rhs,
                            start=(dblk == 0 and term == 0),
                            stop=(dblk == KBLK - 1 and term == nterm - 1),
                        )

                # ---- segment tail ----
                lgT = work.tile([E, tw], f32, name=f"lgT{tw}", bufs=2)
                nc.scalar.copy(lgT[:], lg_ps[:])
                for tbl in range(ntb):
                    tb = tb0 + tbl
                    tp_ps = ptp.tile([128, E], f32, name="tp_ps")
                    nc.tensor.transpose(
                        tp_ps[:], lgT[:, tbl * 128:(tbl + 1) * 128],
                        ident_sb[0:E, :],
                    )
                    nc.vector.tensor_add(
                        biased[:, tb * E:(tb + 1) * E], tp_ps[:], bias_sb[:]
                    )
                for tbl in range(ntb):
                    tb = tb0 + tbl
                    nc.vector.max(mx[:, tb * 8:(tb + 1) * 8],
                                  biased[:, tb * E:(tb + 1) * E])
                    nc.vector.max_index(mi[:, tb * 8:(tb + 1) * 8],
                                        mx[:, tb * 8:(tb + 1) * 8],
                                        biased[:, tb * E:(tb + 1) * E])
                cs = slice(tb0, tb0 + ntb)
                for k in range(2):
                    nc.vector.tensor_copy(idxf[:, k, cs], mi3[:, cs, k])
                masks = []
                for k in range(2):
                    m = work.tile([128, ntb, E], f32, name=f"mask{k}_{tw}",
                                  bufs=2)
                    idxb = idxf[:, k, cs].rearrange(
                        "p (b o) -> p b o", o=1
                    ).broadcast_to((128, ntb, E))
                    nc.vector.tensor_tensor(m[:], bcast3(iota_f, ntb), idxb,
                                            op=Alu.is_equal)
                    masks.append(m)
                md = work.tile([128, ntb, E], f32, name=f"md{tw}", bufs=2)
                nc.vector.tensor_sub(md[:], masks[0][:], masks[1][:])
                nc.vector.tensor_mul(md[:], md[:], bcast3(bias_sb, ntb))
                nc.vector.tensor_reduce(bd[:, cs], md[:],
                                        axis=mybir.AxisListType.X, op=Alu.add)
                nc.vector.tensor_sub(dlt[:, cs], mx3[:, cs, 0], mx3[:, cs, 1])
                nc.vector.tensor_sub(dlt[:, cs], dlt[:, cs], bd[:, cs])
                nc.scalar.activation(w3[:, cs, 0], dlt[:, cs], Act.Sigmoid,
                                     scale=1.0)
                nc.scalar.activation(w3[:, cs, 1], dlt[:, cs], Act.Sigmoid,
                                     scale=-1.0)
                for tbl in range(ntb):
                    tb = tb0 + tbl
                    for k in range(2):
                        nc.tensor.matmul(
                            loads_ps[:],
                            lhsT=w_sb[:, 2 * tb + k:2 * tb + k + 1],
                            rhs=masks[k][:, tbl, :],
                            start=(si == 0 and tbl == 0 and k == 0),
                            stop=(last_seg and tbl == ntb - 1 and k == 1),
                        )

            # ---- indices out (int32) ----
            nc.vector.tensor_copy(
                i_sb[:].rearrange("p (b k) -> p b k", k=2),
                mi3[:, :, 0:2].bitcast(i32),
            )
            loads_sb = work.tile([1, E], f32)
            nc.scalar.copy(loads_sb[:], loads_ps[:])

            # ---- outputs ----
            nc.sync.dma_start(out=w_out.ap(), in_=w_sb[:])
            nc.sync.dma_start(out=i_out.ap(), in_=i_sb[:])
            nc.sync.dma_start(out=loads_out.ap(), in_=loads_sb[:])

        if iters == 1:
            body()
        else:
            with tc.For_i(0, iters, 1,
                          hint_engines=(mybir.EngineType.PE,
                                        mybir.EngineType.DVE,
                                        mybir.EngineType.SP)):
                body()
        ctx.close()
    nc.compile()
    return nc


def make_runner(nc, n_cores=NCORES):
    """Reusable jitted SPMD runner (inputs stay on device across calls)."""
    import jax
    import concourse.mybir as mybir
    from concourse.bass2jax import (
        _bass_exec_p,
        partition_id_tensor,
        install_neuronx_cc_hook,
    )
    from jax.sharding import Mesh, PartitionSpec
    from jax.experimental.shard_map import shard_map

    install_neuronx_cc_hook()
    partition_name = nc.partition_id_tensor.name if nc.partition_id_tensor else None
    in_names, out_names, out_avals = [], [], []
    for alloc in nc.m.functions[0].allocations:
        if not isinstance(alloc, mybir.MemoryLocationSet):
            continue
        name = alloc.memorylocations[0].name
        if alloc.kind == "ExternalInput":
            if name != partition_name:
                in_names.append(name)
        elif alloc.kind == "ExternalOutput":
            out_names.append(name)
            out_avals.append(
                jax.core.ShapedArray(tuple(alloc.tensor_shape),
                                     mybir.dt.np(alloc.dtype))
            )
    n_params = len(in_names)
    n_outs = len(out_avals)
    all_in = list(in_names) + list(out_names)
    if partition_name is not None:
        all_in.append(partition_name)
    donate = tuple(range(n_params, n_params + n_outs))

    def _body(*args):
        operands = list(args)
        if partition_name is not None:
            operands.append(partition_id_tensor())
        return tuple(
            _bass_exec_p.bind(
                *operands,
                out_avals=tuple(out_avals),
                in_names=tuple(all_in),
                out_names=tuple(out_names),
                lowering_input_output_aliases=(),
                sim_require_finite=True,
                sim_require_nnan=True,
                nc=nc,
            )
        )

    devices = jax.devices()[:n_cores]
    mesh = Mesh(np.asarray(devices), ("core",))
    fn = jax.jit(
        shard_map(
            _body,
            mesh=mesh,
            in_specs=(PartitionSpec("core"),) * (n_params + n_outs),
            out_specs=(PartitionSpec("core"),) * n_outs,
            check_rep=False,
        ),
        donate_argnums=donate,
        keep_unused=True,
    )

    def run(in_maps_dev):
        """in_maps_dev: list of per-input device/np arrays already concatenated
        on axis 0 across cores, in in_names order."""
        import jax as _jax

        zeros = [
            _jax.device_put(np.zeros((n_cores * a.shape[0], *a.shape[1:]), a.dtype))
            for a in out_avals
        ]
        outs = fn(*in_maps_dev, *zeros)
        return {
            name: np.asarray(outs[i]).reshape(n_cores, *out_avals[i].shape)
            for i, name in enumerate(out_names)
        }

    return run, in_names, out_names, out_avals, fn, n_params


def _host_prep(x, gate_w, expert_loads, mode="bf16x3"):
    split = mode == "bf16x3"
    nxp = 2 if split else 1
    gw_m = 128 if split else E
    npdt = BF16 if split else np.float32

    # gws [128, KBLK, gw_m]: stationary operand, split mode packs [gh | gl]
    gws = np.empty((128, KBLK, gw_m), dtype=npdt)
    if split:
        gwh = gate_w.astype(BF16)
        gwl = (gate_w - gwh.astype(np.float32)).astype(BF16)
        gws[:, :, 0:E] = gwh.reshape(KBLK, 128, E).transpose(1, 0, 2)
        gws[:, :, E:2 * E] = gwl.reshape(KBLK, 128, E).transpose(1, 0, 2)
    else:
        gws[:, :, :] = (
            gate_w.astype(np.float32).reshape(KBLK, 128, E).transpose(1, 0, 2)
        )

    bias = (-(expert_loads.astype(np.float32) - 1.0 / E) * 2.0).astype(np.float32)
    bias_bc = np.ascontiguousarray(np.broadcast_to(bias, (128, E)))
    ident = np.concatenate([np.eye(64, dtype=np.float32)] * 2, axis=0)

    xs_cores = []
    for c in range(NCORES):
        blkT = x[c * TS:(c + 1) * TS, :].T  # [D, TS] view
        blkT = np.ascontiguousarray(blkT)
        # xs[c, p, d, h, t] = plane_h[d*128+p, c*512+t]
        xsc = np.empty((NCH, 128, KBLK, nxp, 512), dtype=npdt)
        if mode == "bf16x3":
            xh = blkT.astype(BF16)
            xl = (blkT - xh.astype(np.float32)).astype(BF16)
            xsc[:, :, :, 0] = xh.reshape(KBLK, 128, NCH, 512).transpose(2, 1, 0, 3)
            xsc[:, :, :, 1] = xl.reshape(KBLK, 128, NCH, 512).transpose(2, 1, 0, 3)
        else:
            xsc[:, :, :, 0] = blkT.reshape(KBLK, 128, NCH, 512).transpose(2, 1, 0, 3)
        xs_cores.append(xsc)
    return xs_cores, gws, bias_bc, ident, bias


def _get_runner(mode="bf16x3", iters=1, seg_split=True):
    key = (mode, iters, seg_split)
    if key not in _RUNNER_CACHE:
        nc = build_nc(mode=mode, iters=iters, seg_split=seg_split)
        _RUNNER_CACHE[key] = make_runner(nc)
    return _RUNNER_CACHE[key]


def kernel(x, gate_w, expert_loads, mode="bf16x3"):
    x = np.asarray(x, dtype=np.float32)
    gate_w = np.asarray(gate_w, dtype=np.float32)
    expert_loads = np.asarray(expert_loads, dtype=np.float32)

    run, in_names, out_names, out_avals, fn, n_params = _get_runner(mode)
    xs_cores, gws, bias_bc, ident, bias = _host_prep(x, gate_w, expert_loads, mode)

    per_input = {
        "xs": np.concatenate(xs_cores, axis=0),
        "gws": np.concatenate([gws] * NCORES, axis=0),
        "bias_bc": np.concatenate([bias_bc] * NCORES, axis=0),
        "ident": np.concatenate([ident] * NCORES, axis=0),
        "salt": np.zeros((NCORES, _salt_len(mode, 1)), np.float32),
    }
    args = [per_input[name] for name in in_names]
    outs = run(args)

    # ---- unshard ----
    w = outs["w_out"]      # [NCORES, 128, 2*NTB]
    idx = outs["i_out"]    # [NCORES, 128, 2*NTB]
    loads = outs["loads_out"]  # [NCORES, 1, E]

    routing = np.empty((T, 2), dtype=np.float32)
    top_idx = np.empty((T, 2), dtype=np.int32)
    for c in range(NCORES):
        routing[c * TS:(c + 1) * TS] = (
            w[c].reshape(128, NTB, 2).transpose(1, 0, 2).reshape(TS, 2)
        )
        top_idx[c * TS:(c + 1) * TS] = (
            idx[c].reshape(128, NTB, 2).transpose(1, 0, 2).reshape(TS, 2)
        )
    current = loads.sum(axis=0)[0] / np.float32(T)
    new_loads = (
        np.float32(SMOOTHING) * expert_loads + np.float32(1.0 - SMOOTHING) * current
    ).astype(np.float32)
    return routing, top_idx, new_loads


# revision 37
# speedup vs baseline: 1.0654x; 1.0654x over previous
"""Trainium2 Bass kernel for nn_AdaptiveRouter (MoE top-2 routing).

Computes, for x [16384, 2048], gate_w [2048, 64], expert_loads [64]:
  logits = x @ gate_w
  top2 idx of (logits + bias), bias = -(loads - 1/64)*2
  routing_weights = softmax of the selected unbiased logits
  new_loads = 0.9*loads + 0.1*segment_sum(weights)/T

Strategy: data-parallel over tokens across 8 NeuronCores. Host pre-transposes
x to [D, T] layout and splits f32 into bf16 hi+lo pairs (exact to ~2^-17) so
the gate matmul runs as three bf16 matmuls at full PE rate with the
contraction dim on partitions (same total DMA bytes as f32). On device:
matmul -> logits^T [64, T] in PSUM, PE-transpose back to [T, 64] tiles,
top-2 via DVE max8/max_index, softmax via sigmoid of the logit difference
(bias gathered by one-hot masks), per-expert load sums via tiny PE matmuls
contracting the token-partition axis. The 8 per-core load partials are
summed on the host during unsharding (the all-reduce of the hint).
"""

import os
import sys

for _p in ("/opt/trn_rl_repo", "/root/.axon_site/_ro/trn_rl_repo"):
    if os.path.isdir(_p) and _p not in sys.path:
        sys.path.insert(0, _p)

import numpy as np
import ml_dtypes

BF16 = ml_dtypes.bfloat16

T, D, E = 16384, 2048, 64
NCORES = 8
TS = T // NCORES          # tokens per core shard = 2048
KBLK = D // 128           # 16 contraction blocks
NTB = TS // 128           # 16 token blocks per core
NCH = TS // 512           # 4 psum chunks per core
SMOOTHING = 0.9

_RUNNER_CACHE = {}

# Bumped on every kernel-graph change: the NEFF compile cache keys on the HLO,
# which does NOT encode the bass graph (only the I/O signature) — a unique
# salt-input shape per version prevents stale-NEFF cache hits.
KERNEL_VERSION = 20


def _salt_len(mode, iters, seg_split=True, big_mid=False, dq=4):
    return (KERNEL_VERSION * 131 + len(mode) * 7 + (iters % 97)
            + (0 if seg_split else 499) + (211 if big_mid else 0)
            + dq * 53) % 997 + 1


def build_nc(mode="bf16x3", iters=1, seg_split=True, big_mid=False, dq=4):
    import concourse.bacc as bacc
    import concourse.mybir as mybir
    from concourse import tile

    f32 = mybir.dt.float32
    bf16 = mybir.dt.bfloat16
    i32 = mybir.dt.int32
    u32 = mybir.dt.uint32
    Alu = mybir.AluOpType
    Act = mybir.ActivationFunctionType

    split = mode == "bf16x3"
    nterm = 3 if split else 1    # matmul terms: gh@xh, gl@xh, gh@xl
    nxp = 2 if split else 1      # x planes: xh, xl
    gw_m = 128 if split else E   # gws packs [gh | gl] along the last axis
    xdt = bf16 if split else f32
    # xs DRAM layout: [NCH, 128, KBLK, nxp, 512] (chunk-major over tokens,
    # partition-contiguous so one chunk = one large efficient DMA)
    nc = bacc.Bacc("TRN2", target_bir_lowering=False, debug=False, num_devices=NCORES)
    xs = nc.dram_tensor("xs", [NCH, 128, KBLK, nxp, 512], xdt, kind="ExternalInput")
    salt = nc.dram_tensor(
        "salt", [1, _salt_len(mode, iters, seg_split, big_mid, dq)], f32,
        kind="ExternalInput")
    gws = nc.dram_tensor("gws", [128, KBLK, gw_m], xdt, kind="ExternalInput")
    bias_bc = nc.dram_tensor("bias_bc", [128, E], f32, kind="ExternalInput")
    ident = nc.dram_tensor("ident", [128, 64], f32, kind="ExternalInput")
    w_out = nc.dram_tensor("w_out", [128, 2 * NTB], f32, kind="ExternalOutput")
    i_out = nc.dram_tensor("i_out", [128, 2 * NTB], i32, kind="ExternalOutput")
    loads_out = nc.dram_tensor("loads_out", [1, E], f32, kind="ExternalOutput")

    with tile.TileContext(nc) as tc:
        from contextlib import ExitStack

        ctx = ExitStack()
        const = ctx.enter_context(tc.tile_pool(name="const", bufs=1))
        xpool = ctx.enter_context(tc.tile_pool(name="xpool", bufs=4))
        work = ctx.enter_context(tc.tile_pool(name="work", bufs=1))
        pmain = ctx.enter_context(tc.tile_pool(name="pmain", bufs=1, space="PSUM"))
        ptp = ctx.enter_context(tc.tile_pool(name="ptp", bufs=2, space="PSUM"))

        # ---- constants ----
        gws_sb = const.tile([128, KBLK, gw_m], xdt)
        nc.sync.dma_start(out=gws_sb[:], in_=gws.ap())
        bias_sb = const.tile([128, E], f32)
        nc.sync.dma_start(out=bias_sb[:], in_=bias_bc.ap())
        ident_sb = const.tile([128, 64], f32)
        nc.sync.dma_start(out=ident_sb[:], in_=ident.ap())
        iota_i = const.tile([128, E], i32)
        nc.gpsimd.iota(iota_i[:], pattern=[[1, E]], base=0, channel_multiplier=0)
        iota_f = const.tile([128, E], f32)
        nc.vector.tensor_copy(iota_f[:], iota_i[:])


        def body():
            # whole-shard accumulators, written chunk-slice-wise
            biased = work.tile([128, NTB * E], f32)
            mx = work.tile([128, NTB * 8], f32)
            mi = work.tile([128, NTB * 8], u32)
            idxf = work.tile([128, 2, NTB], f32)
            bd = work.tile([128, NTB], f32)
            dlt = work.tile([128, NTB], f32)
            w_sb = work.tile([128, 2 * NTB], f32)
            i_sb = work.tile([128, 2 * NTB], i32)
            loads_ps = pmain.tile([1, E], f32, name="loads_ps")

            mi3 = mi[:].rearrange("p (b s) -> p b s", s=8)
            mx3 = mx[:].rearrange("p (b s) -> p b s", s=8)
            w3 = w_sb[:].rearrange("p (b k) -> p b k", k=2)

            def bcast3(t, nb):
                return t[:].rearrange("p (o e) -> p o e", o=1).broadcast_to(
                    (128, nb, E)
                )

            DQ = dq  # dblks per DMA (DQ/4 MB each, contiguous per partition)
            # All x DMAs are issued up-front into resident tiles: the SDMA
            # queue streams continuously and matmuls chase the arriving quads.
            xtiles = {}
            for c in range(NCH):
                if big_mid and 0 < c < NCH - 1:
                    xt = xpool.tile([128, KBLK, nxp, 512], xdt,
                                    name=f"x{c}", bufs=1)
                    nc.sync.dma_start(out=xt[:], in_=xs.ap()[c])
                    for q in range(KBLK // DQ):
                        xtiles[(c, q)] = None
                    xtiles[(c, "full")] = xt
                    continue
                for q in range(KBLK // DQ):
                    xt = xpool.tile([128, DQ, nxp, 512], xdt,
                                    name=f"x{c}_{q}", bufs=1)
                    nc.sync.dma_start(
                        out=xt[:], in_=xs.ap()[c, :, q * DQ:(q + 1) * DQ]
                    )
                    xtiles[(c, q)] = xt

            # Segments: full 512-token chunks, except the last chunk is split
            # into two 256-token halves so the final exposed tail is shorter.
            if seg_split:
                segs = [(c, 0, 512) for c in range(NCH - 1)]
                segs += [(NCH - 1, 0, 256), (NCH - 1, 256, 256)]
            else:
                segs = [(c, 0, 512) for c in range(NCH)]
            for si, (c, t0, tw) in enumerate(segs):
                ntb = tw // 128
                tb0 = (c * 512 + t0) // 128
                last_seg = si == len(segs) - 1

                # ---- gate matmul for this token segment ----
                lg_ps = pmain.tile([E, tw], f32, name=f"lg_ps{tw}", bufs=2)
                for dblk in range(KBLK):
                    for term in range(nterm):
                        # term 0: gh@xh, term 1: gl@xh, term 2: gh@xl
                        g_lo = E if term == 1 else 0
                        x_i = 1 if term == 2 else 0
                        if xtiles[(c, dblk // DQ)] is None:
                            rhs = xtiles[(c, "full")][:, dblk, x_i, t0:t0 + tw]
                        else:
                            rhs = xtiles[(c, dblk // DQ)][:, dblk % DQ, x_i,
                                                          t0:t0 + tw]
                        nc.tensor.matmul(
                            lg_ps[:],
                            lhsT=gws_sb[:, dblk, g_lo:g_lo + E],
                            rhs=rhs,
                            start=(dblk == 0 and term == 0),
                            stop=(dblk == KBLK - 1 and term == nterm - 1),
                        )

                # ---- segment tail ----
                lgT = work.tile([E, tw], f32, name=f"lgT{tw}", bufs=2)
                nc.scalar.copy(lgT[:], lg_ps[:])
                for tbl in range(ntb):
                    tb = tb0 + tbl
                    tp_ps = ptp.tile([128, E], f32, name="tp_ps")
                    nc.tensor.transpose(
                        tp_ps[:], lgT[:, tbl * 128:(tbl + 1) * 128],
                        ident_sb[0:E, :],
                    )
                    nc.vector.tensor_add(
                        biased[:, tb * E:(tb + 1) * E], tp_ps[:], bias_sb[:]
                    )
                for tbl in range(ntb):
                    tb = tb0 + tbl
                    nc.vector.max(mx[:, tb * 8:(tb + 1) * 8],
                                  biased[:, tb * E:(tb + 1) * E])
                    nc.vector.max_index(mi[:, tb * 8:(tb + 1) * 8],
                                        mx[:, tb * 8:(tb + 1) * 8],
                                        biased[:, tb * E:(tb + 1) * E])
                cs = slice(tb0, tb0 + ntb)
                for k in range(2):
                    nc.vector.tensor_copy(idxf[:, k, cs], mi3[:, cs, k])
                masks = []
                for k in range(2):
                    m = work.tile([128, ntb, E], f32, name=f"mask{k}_{tw}",
                                  bufs=2)
                    idxb = idxf[:, k, cs].rearrange(
                        "p (b o) -> p b o", o=1
                    ).broadcast_to((128, ntb, E))
                    nc.vector.tensor_tensor(m[:], bcast3(iota_f, ntb), idxb,
                                            op=Alu.is_equal)
                    masks.append(m)
                md = work.tile([128, ntb, E], f32, name=f"md{tw}", bufs=2)
                nc.vector.tensor_sub(md[:], masks[0][:], masks[1][:])
                nc.vector.tensor_mul(md[:], md[:], bcast3(bias_sb, ntb))
                nc.vector.tensor_reduce(bd[:, cs], md[:],
                                        axis=mybir.AxisListType.X, op=Alu.add)
                nc.vector.tensor_sub(dlt[:, cs], mx3[:, cs, 0], mx3[:, cs, 1])
                nc.vector.tensor_sub(dlt[:, cs], dlt[:, cs], bd[:, cs])
                nc.scalar.activation(w3[:, cs, 0], dlt[:, cs], Act.Sigmoid,
                                     scale=1.0)
                nc.scalar.activation(w3[:, cs, 1], dlt[:, cs], Act.Sigmoid,
                                     scale=-1.0)
                for tbl in range(ntb):
                    tb = tb0 + tbl
                    for k in range(2):
                        nc.tensor.matmul(
                            loads_ps[:],
                            lhsT=w_sb[:, 2 * tb + k:2 * tb + k + 1],
                            rhs=masks[k][:, tbl, :],
                            start=(si == 0 and tbl == 0 and k == 0),
                            stop=(last_seg and tbl == ntb - 1 and k == 1),
                        )

            # ---- indices out (int32) ----
            nc.vector.tensor_copy(
                i_sb[:].rearrange("p (b k) -> p b k", k=2),
                mi3[:, :, 0:2].bitcast(i32),
            )
            loads_sb = work.tile([1, E], f32)
            nc.scalar.copy(loads_sb[:], loads_ps[:])

            # ---- outputs ----
            nc.sync.dma_start(out=w_out.ap(), in_=w_sb[:])
            nc.sync.dma_start(out=i_out.ap(), in_=i_sb[:])
            nc.sync.dma_start(out=loads_out.ap(), in_=loads_sb[:])

        if iters == 1:
            body()
        else:
            with tc.For_i(0, iters, 1,
                          hint_engines=(mybir.EngineType.PE,
                                        mybir.EngineType.DVE,
                                        mybir.EngineType.SP)):
                body()
        ctx.close()
    nc.compile()
    return nc


def make_runner(nc, n_cores=NCORES):
    """Reusable jitted SPMD runner (inputs stay on device across calls)."""
    import jax
    import concourse.mybir as mybir
    from concourse.bass2jax import (
        _bass_exec_p,
        partition_id_tensor,
        install_neuronx_cc_hook,
    )
    from jax.sharding import Mesh, PartitionSpec
    from jax.experimental.shard_map import shard_map

    install_neuronx_cc_hook()
    partition_name = nc.partition_id_tensor.name if nc.partition_id_tensor else None
    in_names, out_names, out_avals = [], [], []
    for alloc in nc.m.functions[0].allocations:
        if not isinstance(alloc, mybir.MemoryLocationSet):
            continue
        name = alloc.memorylocations[0].name
        if alloc.kind == "ExternalInput":
            if name != partition_name:
                in_names.append(name)
        elif alloc.kind == "ExternalOutput":
            out_names.append(name)
            out_avals.append(
                jax.core.ShapedArray(tuple(alloc.tensor_shape),
                                     mybir.dt.np(alloc.dtype))
            )
    n_params = len(in_names)
    n_outs = len(out_avals)
    all_in = list(in_names) + list(out_names)
    if partition_name is not None:
        all_in.append(partition_name)
    donate = tuple(range(n_params, n_params + n_outs))

    def _body(*args):
        operands = list(args)
        if partition_name is not None:
            operands.append(partition_id_tensor())
        return tuple(
            _bass_exec_p.bind(
                *operands,
                out_avals=tuple(out_avals),
                in_names=tuple(all_in),
                out_names=tuple(out_names),
                lowering_input_output_aliases=(),
                sim_require_finite=True,
                sim_require_nnan=True,
                nc=nc,
            )
        )

    devices = jax.devices()[:n_cores]
    mesh = Mesh(np.asarray(devices), ("core",))
    fn = jax.jit(
        shard_map(
            _body,
            mesh=mesh,
            in_specs=(PartitionSpec("core"),) * (n_params + n_outs),
            out_specs=(PartitionSpec("core"),) * n_outs,
            check_rep=False,
        ),
        donate_argnums=donate,
        keep_unused=True,
    )

    def run(in_maps_dev):
        """in_maps_dev: list of per-input device/np arrays already concatenated
        on axis 0 across cores, in in_names order."""
        import jax as _jax

        zeros = [
            _jax.device_put(np.zeros((n_cores * a.shape[0], *a.shape[1:]), a.dtype))
            for a in out_avals
        ]
        outs = fn(*in_maps_dev, *zeros)
        return {
            name: np.asarray(outs[i]).reshape(n_cores, *out_avals[i].shape)
            for i, name in enumerate(out_names)
        }

    return run, in_names, out_names, out_avals, fn, n_params


def _host_prep(x, gate_w, expert_loads, mode="bf16x3"):
    split = mode == "bf16x3"
    nxp = 2 if split else 1
    gw_m = 128 if split else E
    npdt = BF16 if split else np.float32

    # gws [128, KBLK, gw_m]: stationary operand, split mode packs [gh | gl]
    gws = np.empty((128, KBLK, gw_m), dtype=npdt)
    if split:
        gwh = gate_w.astype(BF16)
        gwl = (gate_w - gwh.astype(np.float32)).astype(BF16)
        gws[:, :, 0:E] = gwh.reshape(KBLK, 128, E).transpose(1, 0, 2)
        gws[:, :, E:2 * E] = gwl.reshape(KBLK, 128, E).transpose(1, 0, 2)
    else:
        gws[:, :, :] = (
            gate_w.astype(np.float32).reshape(KBLK, 128, E).transpose(1, 0, 2)
        )

    bias = (-(expert_loads.astype(np.float32) - 1.0 / E) * 2.0).astype(np.float32)
    bias_bc = np.ascontiguousarray(np.broadcast_to(bias, (128, E)))
    ident = np.concatenate([np.eye(64, dtype=np.float32)] * 2, axis=0)

    xs_cores = []
    for c in range(NCORES):
        blkT = x[c * TS:(c + 1) * TS, :].T  # [D, TS] view
        blkT = np.ascontiguousarray(blkT)
        # xs[c, p, d, h, t] = plane_h[d*128+p, c*512+t]
        xsc = np.empty((NCH, 128, KBLK, nxp, 512), dtype=npdt)
        if mode == "bf16x3":
            xh = blkT.astype(BF16)
            xl = (blkT - xh.astype(np.float32)).astype(BF16)
            xsc[:, :, :, 0] = xh.reshape(KBLK, 128, NCH, 512).transpose(2, 1, 0, 3)
            xsc[:, :, :, 1] = xl.reshape(KBLK, 128, NCH, 512).transpose(2, 1, 0, 3)
        else:
            xsc[:, :, :, 0] = blkT.reshape(KBLK, 128, NCH, 512).transpose(2, 1, 0, 3)
        xs_cores.append(xsc)
    return xs_cores, gws, bias_bc, ident, bias


def _get_runner(mode="bf16x3", iters=1, seg_split=True, big_mid=False, dq=4):
    key = (mode, iters, seg_split, big_mid, dq)
    if key not in _RUNNER_CACHE:
        nc = build_nc(mode=mode, iters=iters, seg_split=seg_split,
                      big_mid=big_mid, dq=dq)
        _RUNNER_CACHE[key] = make_runner(nc)
    return _RUNNER_CACHE[key]


def kernel(x, gate_w, expert_loads, mode="bf16x3"):
    x = np.asarray(x, dtype=np.float32)
    gate_w = np.asarray(gate_w, dtype=np.float32)
    expert_loads = np.asarray(expert_loads, dtype=np.float32)

    run, in_names, out_names, out_avals, fn, n_params = _get_runner(mode)
    xs_cores, gws, bias_bc, ident, bias = _host_prep(x, gate_w, expert_loads, mode)

    per_input = {
        "xs": np.concatenate(xs_cores, axis=0),
        "gws": np.concatenate([gws] * NCORES, axis=0),
        "bias_bc": np.concatenate([bias_bc] * NCORES, axis=0),
        "ident": np.concatenate([ident] * NCORES, axis=0),
        "salt": np.zeros((NCORES, _salt_len(mode, 1)), np.float32),
    }
    args = [per_input[name] for name in in_names]
    outs = run(args)

    # ---- unshard ----
    w = outs["w_out"]      # [NCORES, 128, 2*NTB]
    idx = outs["i_out"]    # [NCORES, 128, 2*NTB]
    loads = outs["loads_out"]  # [NCORES, 1, E]

    routing = np.empty((T, 2), dtype=np.float32)
    top_idx = np.empty((T, 2), dtype=np.int32)
    for c in range(NCORES):
        routing[c * TS:(c + 1) * TS] = (
            w[c].reshape(128, NTB, 2).transpose(1, 0, 2).reshape(TS, 2)
        )
        top_idx[c * TS:(c + 1) * TS] = (
            idx[c].reshape(128, NTB, 2).transpose(1, 0, 2).reshape(TS, 2)
        )
    current = loads.sum(axis=0)[0] / np.float32(T)
    new_loads = (
        np.float32(SMOOTHING) * expert_loads + np.float32(1.0 - SMOOTHING) * current
    ).astype(np.float32)
    return routing, top_idx, new_loads
